# revision 22
# baseline (speedup 1.0000x reference)
"""Form-B GNN attention kernel: input2 as stationary matmul operand (f32r),
attention tiles as bf16 moving operands, adj.T via DMA xbar transpose.

Per row-block (512 rows) the accumulators live transposed in PSUM:
  o1T[d, row] = sum_j in2[j, d] * attm[j, row]     (attm = adj * exp(lrelu(e)))
  o2T[d, row] = sum_j in2[j, d] * adjT[j, row]
  Zrow[1, row], degrow[1, row] via ones-column lhsT.
Tail: c1 = 0.1*deg/(Z+eps) broadcast via PE; out = c1*o1T + 0.9*o2T,
re-transposed to natural [row, d] and DMA'd out.
"""

import numpy as np
from contextlib import ExitStack

import concourse.bass as bass
import concourse.bacc as bacc
import concourse.tile as tile
from concourse import mybir
from concourse.masks import make_identity
from concourse.bass_utils import run_bass_kernel_spmd

F32 = mybir.dt.float32
F32R = mybir.dt.float32r
F16 = mybir.dt.float16
EXP_SHIFT = 8.0

N_CORES = 8
N, M, D = 8192, 8192, 256
GAMMA = 0.1
P = 128


def build_kernel(nc, tc, ctx, rows, m, d, rb_rows=512, jload=1024):
    assert rows % rb_rows == 0 and rb_rows % P == 0 and m % jload == 0
    assert jload % P == 0 and d == 256
    K = rb_rows // P           # 128-row chunks per row-block
    NRB = rows // rb_rows
    JC = m // P
    JPG = jload // P

    input1 = nc.dram_tensor("input1s", [rows, d], F32, kind="ExternalInput").ap()
    input2 = nc.dram_tensor("input2", [m, d], F32, kind="ExternalInput").ap()
    adjT_d = nc.dram_tensor("adjTs", [m, rows], F16, kind="ExternalInput").ap()
    a1b = nc.dram_tensor("a1b", [P, d], F32, kind="ExternalInput").ap()
    a2b = nc.dram_tensor("a2b", [P, d], F32, kind="ExternalInput").ap()
    out = nc.dram_tensor("outs", [(rows // rb_rows) * d, rb_rows], F32,
                         kind="ExternalOutput").ap()

    const_pool = ctx.enter_context(tc.tile_pool(name="const", bufs=1))
    in2_pool = ctx.enter_context(tc.tile_pool(name="in2", bufs=1))
    adj_pool = ctx.enter_context(tc.tile_pool(name="adjnat", bufs=6))
    work_pool = ctx.enter_context(tc.tile_pool(name="work", bufs=4))
    junk_pool = ctx.enter_context(tc.tile_pool(name="junk", bufs=1))
    out_pool = ctx.enter_context(tc.tile_pool(name="outp", bufs=2))
    tail_pool = ctx.enter_context(tc.tile_pool(name="tail", bufs=1))

    ps_acc = ctx.enter_context(tc.tile_pool(name="ps_acc", bufs=1, space="PSUM"))
    ps_stat = ctx.enter_context(tc.tile_pool(name="ps_stat", bufs=1, space="PSUM"))
    ps_misc = ctx.enter_context(tc.tile_pool(name="ps_misc", bufs=1, space="PSUM"))
    ps_stage = ctx.enter_context(tc.tile_pool(name="ps_stage", bufs=1, space="PSUM"))

    # ---- constants ----
    identity = const_pool.tile([P, P], F32, tag="identity")
    make_identity(nc, identity[:])
    ones_b = const_pool.tile([P, 1], F16, tag="ones_b")
    nc.vector.memset(ones_b[:], 1.0)
    negc = const_pool.tile([P, 1], F32, tag="negc")
    nc.vector.memset(negc[:], -EXP_SHIFT)
    ones1 = const_pool.tile([1, P], F32, tag="ones1")
    nc.vector.memset(ones1[:], 1.0)
    a1b_sb = const_pool.tile([P, d], F32, tag="a1b")
    nc.sync.dma_start(out=a1b_sb[:], in_=a1b)
    a2b_sb = const_pool.tile([P, d], F32, tag="a2b")
    nc.sync.dma_start(out=a2b_sb[:], in_=a2b)

    # ---- input1 + e1 first: e1b gates the very first attention tile ----
    T1 = rows // P
    in1_sb = const_pool.tile([P, T1, d], F32, tag="in1sb")
    nc.sync.dma_start(out=in1_sb[:], in_=input1.rearrange("(t p) d -> p t d", p=P))
    e1_sb = const_pool.tile([P, T1], F32, tag="e1")
    for t in range(T1):
        jt = junk_pool.tile([P, d], F32, tag="junk")
        nc.vector.tensor_mul(jt[:], in1_sb[:, t, :], a1b_sb[:])
        nc.vector.reduce_sum(e1_sb[:, t:t + 1], jt[:], axis=mybir.AxisListType.X)

    # ---- input2 -> exact fp16 hi/lo split (weights), f32 staged in chunks ----
    in2_hi = in2_pool.tile([P, JC, d], F16, tag="in2hi")
    in2_lo = in2_pool.tile([P, JC, d], F16, tag="in2lo")
    e2_sb = const_pool.tile([P, JC], F32, tag="e2")
    a2b_sb_ref = a2b_sb
    in2_r = input2.rearrange("(t p) d -> p t d", p=P)
    G = max(1, JC // 8)
    step = JC // G
    for g in range(G):
        stg = in2_pool.tile([P, step, d], F32, tag="in2stg", bufs=3,
                            name=f"in2stg_{g}")
        nc.sync.dma_start(out=stg[:], in_=in2_r[:, g * step:(g + 1) * step, :])
        gs = slice(g * step, (g + 1) * step)
        nc.vector.tensor_copy(in2_hi[:, gs, :], stg[:])
        nc.vector.scalar_tensor_tensor(
            out=in2_lo[:, gs, :], in0=stg[:], scalar=1.0, in1=in2_hi[:, gs, :],
            op0=mybir.AluOpType.mult, op1=mybir.AluOpType.subtract,
        )
        jt = junk_pool.tile([P, step, d], F32, tag="junk", name=f"jt2_{g}")
        # a2b broadcast across the chunk axis via a 0-stride AP
        nc.vector.tensor_mul(jt[:], stg[:], a2b_sb[:].rearrange('p (o d) -> p o d', o=1).broadcast_to((P, step, d)))
        nc.vector.reduce_sum(e2_sb[:, gs], jt[:], axis=mybir.AxisListType.X)

    out_r = out.rearrange("(b c p) f -> b c p f", c=d // P, p=P)
    adjT_r = adjT_d.rearrange("(g s p) (b f) -> g b p s f", p=P, s=JPG, f=rb_rows)

    for rb in range(NRB):
        # E1B broadcast: e1 col -> [1,128] psum rows -> e1row -> one K=1 matmul
        e1row = tail_pool.tile([1, rb_rows], F32, tag="e1row")
        for c in range(K):
            tp = ps_misc.tile([1, P], F32, tag="misc", name=f"e1t_{rb}_{c}")
            nc.tensor.transpose(tp[:], e1_sb[:, rb * K + c:rb * K + c + 1], identity[:])
            nc.scalar.copy(e1row[:, c * P:(c + 1) * P], tp[:])
        e1b_ps = ps_misc.tile([P, rb_rows], F32, tag="misc", name=f"e1b_{rb}")
        nc.tensor.matmul(e1b_ps[:], ones1[:], e1row[:], start=True, stop=True)
        e1b = work_pool.tile([P, rb_rows], F32, tag="e1b", bufs=1)
        nc.scalar.copy(e1b[:], e1b_ps[:])

        o1T = [ps_acc.tile([P, rb_rows], F32, tag=f"o1T{c}", name=f"o1T{c}_{rb}") for c in range(2)]
        o2T = [ps_acc.tile([P, rb_rows], F32, tag=f"o2T{c}", name=f"o2T{c}_{rb}") for c in range(2)]
        zrow = ps_stat.tile([1, rb_rows], F32, tag="zrow", name=f"zrow_{rb}")
        drow = ps_stat.tile([1, rb_rows], F32, tag="drow", name=f"drow_{rb}")

        for jc in range(JC):
            jg, jo = divmod(jc, JPG)
            if jo == 0:
                # adj ships pre-transposed (f16): plain contiguous load of
                # adjT_big[p=j%128, s=j//128, f=row]
                adjT_big = adj_pool.tile([P, JPG, rb_rows], F16, tag="adjTb",
                                         name=f"adjTb_{rb}_{jg}")
                nc.sync.dma_start(out=adjT_big[:], in_=adjT_r[jg, rb])
            adjT = adjT_big[:, jo, :]

            lr = work_pool.tile([P, rb_rows], F32, tag="lr")
            nc.scalar.activation(
                lr[:], e1b[:], mybir.ActivationFunctionType.Prelu,
                bias=e2_sb[:, jc:jc + 1], scale=1.0, alpha=0.2,
            )
            ex = work_pool.tile([P, rb_rows], F16, tag="ex")
            nc.scalar.activation(ex[:], lr[:], mybir.ActivationFunctionType.Exp,
                                 bias=negc[:])
            attm = work_pool.tile([P, rb_rows], F16, tag="attm")
            nc.vector.tensor_mul(attm[:], ex[:], adjT)

            first, last = jc == 0, jc == JC - 1
            hi0, hi1 = in2_hi[:, jc, 0:P], in2_hi[:, jc, P:d]
            lo0, lo1 = in2_lo[:, jc, 0:P], in2_lo[:, jc, P:d]
            nc.tensor.matmul(o1T[0][:], hi0, attm[:], start=first, stop=last)
            nc.tensor.matmul(o2T[0][:], hi0, adjT, start=first, stop=False)
            nc.tensor.matmul(o2T[0][:], lo0, adjT, start=False, stop=last)
            nc.tensor.matmul(o1T[1][:], hi1, attm[:], start=first, stop=last)
            nc.tensor.matmul(o2T[1][:], hi1, adjT, start=first, stop=False)
            nc.tensor.matmul(o2T[1][:], lo1, adjT, start=False, stop=last)
            nc.tensor.matmul(zrow[:], ones_b[:], attm[:], start=first, stop=last)
            nc.tensor.matmul(drow[:], ones_b[:], adjT, start=first, stop=last)

        # ---- tail ----
        zeps = tail_pool.tile([1, rb_rows], F32, tag="zeps")
        nc.vector.tensor_scalar_add(zeps[:], zrow[:], 1e-30)
        rz = tail_pool.tile([1, rb_rows], F32, tag="rz")
        nc.vector.reciprocal(rz[:], zeps[:])
        c1row = tail_pool.tile([1, rb_rows], F32, tag="c1row")
        nc.vector.scalar_tensor_tensor(
            out=c1row[:], in0=drow[:], scalar=GAMMA, in1=rz[:],
            op0=mybir.AluOpType.mult, op1=mybir.AluOpType.mult,
        )
        c1b_ps = ps_misc.tile([P, rb_rows], F32, tag="misc", name=f"c1b_{rb}")
        nc.tensor.matmul(c1b_ps[:], ones1[:], c1row[:], start=True, stop=True)
        c1b = tail_pool.tile([P, rb_rows], F32, tag="c1b")
        nc.scalar.copy(c1b[:], c1b_ps[:])

        for c in range(2):
            # comb = c1 * o1T + 0.9 * o2T, kept transposed [d-chunk, rows];
            # the host gather re-naturalizes the layout (free during unshard)
            comb = out_pool.tile([P, rb_rows], F32, tag="comb", name=f"comb_{rb}_{c}")
            nc.vector.tensor_mul(comb[:], o1T[c][:], c1b[:])
            t2 = tail_pool.tile([P, rb_rows], F32, tag="t2")
            nc.scalar.mul(t2[:], o2T[c][:], 1.0 - GAMMA)
            nc.vector.tensor_add(comb[:], comb[:], t2[:])
            nc.sync.dma_start(out=out_r[rb, c], in_=comb[:])


def build_nc(rows=N // N_CORES, m=M, d=D, rb_rows=512, jload=1024):
    nc = bacc.Bacc("TRN2", debug=False)
    with tile.TileContext(nc) as tc:
        with ExitStack() as ctx:
            build_kernel(nc, tc, ctx, rows, m, d, rb_rows, jload)
    nc.compile()
    return nc


def kernel(input1, input2, adj, a1, a2, _trace=False):
    rows = input1.shape[0] // N_CORES
    nc = build_nc(rows=rows, m=input2.shape[0], d=input2.shape[1])
    a1b = np.ascontiguousarray(np.broadcast_to(a1.reshape(1, -1), (P, a1.shape[0]))).astype(np.float32)
    a2b = np.ascontiguousarray(np.broadcast_to(a2.reshape(1, -1), (P, a2.shape[0]))).astype(np.float32)
    in_maps = [
        {
            "input1s": np.ascontiguousarray(input1[c * rows:(c + 1) * rows]),
            "input2": np.ascontiguousarray(input2),
            "adjTs": np.ascontiguousarray(
                adj[c * rows:(c + 1) * rows].T).astype(np.float16),
            "a1b": a1b,
            "a2b": a2b,
        }
        for c in range(N_CORES)
    ]
    res = run_bass_kernel_spmd(nc, in_maps, list(range(N_CORES)), trace=_trace)
    RB = 512
    shards = []
    for c in range(N_CORES):
        ot = res.results[c]["outs"].reshape(rows // RB, 2, P, RB)
        shards.append(np.transpose(ot, (0, 3, 1, 2)).reshape(rows, 2 * P))
    out = np.concatenate(shards, axis=0)
    if _trace:
        return out, res
    return out
